# revision 1
# baseline (speedup 1.0000x reference)
"""Bass/Trainium2 kernel for BayesianDropoutLayer:
    out = X @ (mask[:, None] * M) + m
  X [8192, 2048] f32, M [2048, 2048] f32, m [2048] f32, mask [2048] i32.

Strategy: data-parallel over the batch dim across 8 NeuronCores. Each core
computes a [1024, 2048] output shard:
  - X shard is fed host-transposed (XT = X_shard.T, [2048, 1024]) so the
    contraction dim lands on SBUF partitions for the TensorEngine.
  - mask is applied on-device to the XT tiles as a per-partition scalar
    multiply (X @ diag(mask) @ M == (X * mask^T) @ M).
  - fp32r matmuls (full-rate fp32 path, N=512 moving dim) accumulate over
    16 k-tiles into PSUM; bias m is added during PSUM->SBUF eviction.
  - M / m / mask are replicated to every core; outputs are concatenated on
    the host (no collectives needed).
"""

import sys

if "/opt/trn_rl_repo" not in sys.path:
    sys.path.insert(0, "/opt/trn_rl_repo")

import numpy as np

import concourse.bass as bass  # noqa: F401  (registers sys modules)
import concourse.mybir as mybir
import concourse.tile as tile
from concourse import bacc
from concourse.bass_utils import run_bass_kernel_spmd

P = 128
BATCH = 8192
N_IN = 2048          # contraction dim K
UNITS = 2048         # output free dim N
N_CORES = 8
B_SHARD = BATCH // N_CORES          # 1024 rows per core
KT = N_IN // P                      # 16 k-tiles
NP_ = 512                           # units panel width (PSUM bank = 512 f32)
N_PANELS = UNITS // NP_             # 4
BT = B_SHARD // P                   # 8 batch tiles per core

F32 = mybir.dt.float32
F32R = mybir.dt.float32r
I32 = mybir.dt.int32

_CACHED_NC = None


def _build_nc():
    global _CACHED_NC
    if _CACHED_NC is not None:
        return _CACHED_NC

    nc = bacc.Bacc("TRN2", target_bir_lowering=False, debug=False)

    xt = nc.dram_tensor("xt", [N_IN, B_SHARD], F32R, kind="ExternalInput")
    mw = nc.dram_tensor("mw", [N_IN, UNITS], F32R, kind="ExternalInput")
    biasd = nc.dram_tensor("biasd", [1, UNITS], F32, kind="ExternalInput")
    mask2d = nc.dram_tensor("mask2d", [P, KT], I32, kind="ExternalInput")
    # panel-major output layout: each [128, 512] store is a fully contiguous
    # 256 KiB block (2KB-line strided writes only reach ~119 GB/s vs ~289
    # for contiguous); the host re-interleaves panels when gathering.
    out = nc.dram_tensor("out", [N_PANELS, B_SHARD, NP_], F32, kind="ExternalOutput")

    xt3 = xt.rearrange("(kt p) b -> p kt b", p=P)
    mw3 = mw.rearrange("(kt p) n -> p kt n", p=P)

    with tile.TileContext(nc) as tc:
        with (
            tc.tile_pool(name="xtp", bufs=1) as xtp,
            tc.tile_pool(name="mwp", bufs=2) as mwp,
            tc.tile_pool(name="misc", bufs=1) as misc,
            tc.tile_pool(name="outp", bufs=4) as outp,
            tc.tile_pool(name="psum", bufs=8, space="PSUM") as psump,
        ):
            # mask + bias vector ride the Scalar engine's HWDGE queue so they
            # don't delay the Sync queue's bulk loads (and vice versa)
            mask_i = misc.tile([P, KT], I32)
            nc.scalar.dma_start(mask_i[:], mask2d[:, :])
            mask_f = misc.tile([P, KT], F32)
            nc.vector.tensor_copy(mask_f[:], mask_i[:])
            bias_sb = misc.tile([1, UNITS], F32)
            nc.scalar.dma_start(bias_sb[:], biasd[0:1, :])
            ones = misc.tile([1, P], F32)
            nc.vector.memset(ones[:], 1.0)

            # Head: interleave panel-0 M k-batches with XT k-tiles so both
            # operands of contraction step k arrive together and the PE can
            # start as soon as k=0 lands (any output tile needs ALL of XT +
            # one M panel = 12 MiB, so progressive arrival is what hides it).
            # All M loads are 4-k-tile batches (1 MiB per trigger, the Sync
            # engine pays ~0.7us per DMA trigger; bufs=12 = 3 panels of
            # prefetch depth).
            def mw_batch(pn, g):
                n0 = pn * NP_
                t = mwp.tile(
                    [P, 4, NP_], F32R, tag="mwg", bufs=12, name=f"mwg{pn}_{g}"
                )
                nc.sync.dma_start(t[:], mw3[:, 4 * g : 4 * g + 4, n0 : n0 + NP_])
                return t

            xt_tiles = []
            mw_g0 = []

            def xt_load(kt):
                x = xtp.tile([P, B_SHARD], F32R, name=f"xt_{kt}")
                nc.sync.dma_start(x[:], xt3[:, kt, :])
                nc.vector.tensor_scalar_mul(x[:], x[:], mask_f[:, kt : kt + 1])
                xt_tiles.append(x)

            for g in range(KT // 4):
                mw_g0.append(mw_batch(0, g))
                for kt in range(4 * g, 4 * g + 4):
                    xt_load(kt)

            mw_panels = {0: [mw_g0[kt // 4][:, kt % 4, :] for kt in range(KT)]}
            for pn in range(1, N_PANELS):
                mw_g = [mw_batch(pn, g) for g in range(KT // 4)]
                mw_panels[pn] = [mw_g[kt // 4][:, kt % 4, :] for kt in range(KT)]

            # bias broadcast [P, UNITS] built with K=1 matmuls from the 8 KB
            # bias vector (a stride-0 broadcast DMA would put 1 MiB of HBM
            # reads on the critical head); runs in early PE-idle time and
            # warms the PE clock.
            m_bc = misc.tile([P, UNITS], F32)
            for j in range(N_PANELS):
                pb = psump.tile([P, NP_], F32, tag="ps", name=f"psb_{j}")
                nc.tensor.matmul(
                    pb[:],
                    ones[:],
                    bias_sb[0:1, j * NP_ : (j + 1) * NP_],
                    start=True,
                    stop=True,
                )
                nc.vector.tensor_copy(m_bc[:, j * NP_ : (j + 1) * NP_], pb[:])

            for pn in range(N_PANELS):
                n0 = pn * NP_
                mw_tiles = mw_panels[pn]

                for bt in range(BT):
                    b0 = bt * P
                    ps = psump.tile([P, NP_], F32)
                    for kt in range(KT):
                        nc.tensor.matmul(
                            ps[:],
                            xt_tiles[kt][:, b0 : b0 + P],
                            mw_tiles[kt][:],
                            start=(kt == 0),
                            stop=(kt == KT - 1),
                        )
                    ob = outp.tile([P, NP_], F32)
                    nc.vector.tensor_tensor(
                        ob[:], ps[:], m_bc[:, n0 : n0 + NP_], mybir.AluOpType.add
                    )
                    # stores go out on the (otherwise idle) Scalar engine's
                    # HWDGE queue so they don't serialize behind loads on Sync
                    nc.scalar.dma_start(out[pn, b0 : b0 + P, :], ob[:])

    nc.compile()
    _CACHED_NC = nc
    return nc


def _make_in_maps(X, M, m, mask):
    X = np.ascontiguousarray(X, dtype=np.float32)
    M = np.ascontiguousarray(M, dtype=np.float32)
    bias = np.ascontiguousarray(m, dtype=np.float32).reshape(1, UNITS)
    mask2d = np.ascontiguousarray(
        np.asarray(mask, dtype=np.int32).reshape(KT, P).T
    )
    in_maps = []
    for c in range(N_CORES):
        xs = X[c * B_SHARD : (c + 1) * B_SHARD]
        xt = np.ascontiguousarray(xs.T)
        in_maps.append({"xt": xt, "mw": M, "biasd": bias, "mask2d": mask2d})
    return in_maps


def run_sharded(X, M, m, mask, trace=False, trace_cores=None):
    """Returns (full_output, BassKernelResults)."""
    nc = _build_nc()
    in_maps = _make_in_maps(X, M, m, mask)
    res = run_bass_kernel_spmd(
        nc,
        in_maps,
        list(range(N_CORES)),
        trace=trace,
        trace_cores=trace_cores,
    )
    shards = [
        np.ascontiguousarray(r["out"].transpose(1, 0, 2)).reshape(B_SHARD, UNITS)
        for r in res.results
    ]
    out = np.concatenate(shards, axis=0)
    return out, res


def kernel(X, M, m, mask):
    out, _ = run_sharded(X, M, m, mask)
    return out



# revision 8
# speedup vs baseline: 1.2092x; 1.2092x over previous
"""Bass/Trainium2 kernel for BayesianDropoutLayer:
    out = X @ (mask[:, None] * M) + m
  X [8192, 2048] f32, M [2048, 2048] f32, m [2048] f32, mask [2048] i32.

Strategy: 2D sharding — batch 4-way x units 2-way across 8 NeuronCores.
Core c computes out[cb*2048:(cb+1)*2048, cu*1024:(cu+1)*1024] where
cb = c % 4, cu = c // 4.

Host-side prep (not HW-timed):
  - masked-out rows of the contraction are DROPPED entirely (X columns /
    M rows where mask==0 contribute nothing), shrinking K from 2048 to
    ~1844; K is then padded up to a multiple of 128.
  - the bias is folded in as one extra contraction row: X' gets an
    all-ones row, W' gets the bias vector, so out = X'@W' needs no
    separate bias add on device.
  - inputs are pre-tiled so every DMA is a single trigger with >=4KB
    contiguous lines per partition.

Device kernel (per core): 16 accumulation chains (one per 128-row batch
block), each a chain of KT compound matmuls with a [128, 1024] moving
operand spanning 2 PSUM banks — one LDWEIGHTS (the fp32r stationary
reload, ~230ns) is amortized over 1024 moving rows instead of 512,
keeping the PE at its 1 row/cycle fp32r streaming rate. 4 chains run
concurrently (8 PSUM banks); eviction PSUM->SBUF alternates between the
Vector and Scalar engines and stores ride the Scalar DMA queue while
loads own the Sync queue.
"""

import sys

if "/opt/trn_rl_repo" not in sys.path:
    sys.path.insert(0, "/opt/trn_rl_repo")

import numpy as np

import concourse.bass as bass  # noqa: F401  (registers sys modules)
import concourse.mybir as mybir
import concourse.tile as tile
from concourse import bacc
from concourse.bass_utils import run_bass_kernel_spmd

P = 128
BATCH = 8192
N_IN = 2048          # contraction dim K (before mask-drop)
UNITS = 2048
N_CORES = 8
CB = 4               # batch shards
CU = 2               # unit shards
B_CORE = BATCH // CB          # 2048 batch rows per core
NU = UNITS // CU              # 1024 units per core
BT = B_CORE // P              # 16 accumulation chains per core
NW = 4                        # chains per wave (8 PSUM banks / 2 per chain)

F32 = mybir.dt.float32
F32R = mybir.dt.float32r

_CACHED = {}


def _build_nc(KT):
    if KT in _CACHED:
        return _CACHED[KT]

    K = KT * P
    nc = bacc.Bacc("TRN2", target_bir_lowering=False, debug=False)

    xt_d = nc.dram_tensor("xt", [BT, P, K], F32R, kind="ExternalInput")
    w_d = nc.dram_tensor("w", [KT, P, NU], F32R, kind="ExternalInput")
    out_d = nc.dram_tensor("out", [BT, P, NU], F32, kind="ExternalOutput")

    # SBUF budget per partition (~208 KiB usable): xt bufs are KT*0.5 KiB
    # each, w tiles 4 KiB each, out staging 4x4 KiB + warm 2 KiB.
    xt_bufs = min(BT, int((200 - 4 * KT - 16 - 2) / (KT * 0.5)))

    with tile.TileContext(nc) as tc:
        with (
            tc.tile_pool(name="xtp", bufs=xt_bufs) as xtp,
            tc.tile_pool(name="wp", bufs=KT) as wp,
            tc.tile_pool(name="misc", bufs=1) as misc,
            tc.tile_pool(name="outp", bufs=4) as outp,
            tc.tile_pool(name="psum", bufs=4, space="PSUM") as psump,
        ):
            # PE p-state warmup: ramp the tensor-engine clock during the
            # DMA-only head so real matmuls start at full speed. fp32
            # matmuls run 4 cycles/row, so two of them cover the ~4us head.
            warm_src = misc.tile([P, 512], F32)
            nc.vector.memset(warm_src[:], 0.0)
            scratch = psump.tile([P, 512], F32, tag="ps", bufs=8, name="scratch")
            for _ in range(2):
                nc.tensor.matmul(
                    scratch[:],
                    warm_src[:, 0:P],
                    warm_src[:],
                    start=True,
                    stop=True,
                )

            w_tiles = [None] * KT
            xt_tiles = [None] * BT

            def load_w(kt):
                t = wp.tile([P, NU], F32R, tag="w", bufs=KT, name=f"w_{kt}")
                nc.sync.dma_start(t[:], w_d[kt, :, :])
                w_tiles[kt] = t

            def load_xt(bt):
                t = xtp.tile([P, K], F32R, tag="xt", bufs=xt_bufs, name=f"xt_{bt}")
                nc.sync.dma_start(t[:], xt_d[bt, :, :])
                xt_tiles[bt] = t

            # wave 0 needs xt blocks 0-3 and every w tile; interleave so
            # chain 0 can start after just w0+xt0 (~1.4 MiB).
            for i in range(min(4, KT)):
                load_w(i)
                load_xt(i)
            for kt in range(4, KT):
                load_w(kt)
            for bt in range(4, BT):
                load_xt(bt)

            # Each batch chain uses two 1-bank PSUM tiles (unit halves); the
            # two matmuls per (bt, kt) share the same stationary xt slice so
            # only one LDWEIGHTS reload is needed per pair.
            H = NU // 512  # 2 unit halves
            for wv in range(BT // NW):
                pss = [
                    [
                        psump.tile(
                            [P, 512], F32, tag="ps", bufs=8, name=f"ps_{wv}_{i}_{h}"
                        )
                        for h in range(H)
                    ]
                    for i in range(NW)
                ]
                for kt in range(KT):
                    for i in range(NW):
                        bt = wv * NW + i
                        for h in range(H):
                            nc.tensor.matmul(
                                pss[i][h][:],
                                xt_tiles[bt][:, kt * P : (kt + 1) * P],
                                w_tiles[kt][:, h * 512 : (h + 1) * 512],
                                start=(kt == 0),
                                stop=(kt == KT - 1),
                            )
                obs = []
                for i in range(NW):
                    bt = wv * NW + i
                    ob = outp.tile([P, NU], F32, tag="ob", bufs=4, name=f"ob_{bt}")
                    nc.vector.tensor_copy(ob[:, 0:512], pss[i][0][:])
                    nc.scalar.copy(ob[:, 512:1024], pss[i][1][:])
                    obs.append(ob)
                for i in range(NW):
                    bt = wv * NW + i
                    nc.scalar.dma_start(out_d[bt, :, :], obs[i][:])

    nc.compile()
    _CACHED[KT] = nc
    return nc


def _prep_inputs(X, M, m, mask):
    X = np.ascontiguousarray(X, dtype=np.float32)
    M = np.ascontiguousarray(M, dtype=np.float32)
    m = np.asarray(m, dtype=np.float32).reshape(UNITS)
    kept = np.flatnonzero(np.asarray(mask) != 0)
    nk = kept.size
    KT = max(1, (nk + 1 + P - 1) // P)
    K = KT * P

    XT_pad = np.zeros((K, BATCH), np.float32)
    XT_pad[:nk] = X.T[kept]
    XT_pad[nk] = 1.0
    W_pad = np.zeros((K, UNITS), np.float32)
    W_pad[:nk] = M[kept]
    W_pad[nk] = m

    xt_blocks = []
    for cb in range(CB):
        A = XT_pad[:, cb * B_CORE : (cb + 1) * B_CORE].reshape(KT, P, BT, P)
        xt_blocks.append(
            np.ascontiguousarray(A.transpose(2, 1, 0, 3)).reshape(BT, P, K)
        )
    w_halves = [
        np.ascontiguousarray(W_pad[:, cu * NU : (cu + 1) * NU]).reshape(KT, P, NU)
        for cu in range(CU)
    ]
    in_maps = [
        {"xt": xt_blocks[c % CB], "w": w_halves[c // CB]} for c in range(N_CORES)
    ]
    return in_maps, KT


def run_sharded(X, M, m, mask, trace=False, trace_cores=None):
    """Returns (full_output, BassKernelResults)."""
    in_maps, KT = _prep_inputs(X, M, m, mask)
    nc = _build_nc(KT)
    res = run_bass_kernel_spmd(
        nc,
        in_maps,
        list(range(N_CORES)),
        trace=trace,
        trace_cores=trace_cores,
    )
    out = np.empty((BATCH, UNITS), np.float32)
    for c in range(N_CORES):
        cb, cu = c % CB, c // CB
        out[cb * B_CORE : (cb + 1) * B_CORE, cu * NU : (cu + 1) * NU] = (
            res.results[c]["out"].reshape(B_CORE, NU)
        )
    return out, res


def kernel(X, M, m, mask):
    out, _ = run_sharded(X, M, m, mask)
    return out
